# revision 11
# baseline (speedup 1.0000x reference)
"""GCN encoder (concat-edges GCNConv) as a distributed Bass/Tile kernel on 8 NeuronCores.

Strategy (see sharding hint): nodes/output sharded 8 ways; edges partitioned by
destination-node owner; per-core replicated bf16 feature table via AllGather;
remote source features fetched with bulk dma_gather (one SWDGE instruction per
~4096 rows, amortizing the ~1us fixed descriptor-generation cost).

dma_gather needs int16 indices and 256B elements, so the h' table rows are
padded to 128 bf16 (only the first 32 are ever read) and the node space is cut
into 4 buckets of 32512 rows; edge slots are laid out bucket-major so every
gather call indexes within a single bucket.

Math:  out = dinv * (S @ (dinv * (x@W))) + dinv^2*(x@W) + b   with S the real-edge
adjacency (dst<-src sum) and dinv = rsqrt(deg+1); self-loops handled analytically.

Device does the FP math (bf16 cast, h=x@W, deg=hi-lo, rsqrt, prescale, mask build,
aggregation matmuls, final scale+bias). Host only does layout: int64->int32, edge
bucketing/sorting by destination, slot/padding assignment, dst-run prefix offsets,
transposes.
"""
import sys

if "/opt/trn_rl_repo" not in sys.path:
    sys.path.insert(0, "/opt/trn_rl_repo")

import numpy as np
import ml_dtypes

P = 128          # SBUF partitions / PE contraction size
LAT = 32         # latent size
IN = 128         # in channels
EPAD = 128       # padded h' row length (bf16) -> 256B elements for dma_gather
BUCK = 32512     # bucket size in table rows (int16 index limit)
MC = 8           # tiles per mask-build instruction
GCH = 7          # tiles per dma_gather call (896 rows; SWDGE ring holds
                 # ~64 descs/queue/direction -> num_idxs/16+1 must stay <63)
G_BUFS = 8
MASK_BUFS = 10


def _full_cfg():
    return dict(N=100_000, NC=8, SH=12_544)  # SH*NC = 100352 >= N, SH % 128 == 0


# ---------------------------------------------------------------- host layout
def prepare(x, edge_index, y_edge_index, W, b, cfg):
    N, NC, SH = cfg["N"], cfg["NC"], cfg["SH"]
    NG = SH // P                    # 128-node dst groups per core
    NPAD = NC * SH
    NQ = (NPAD + BUCK - 1) // BUCK  # src buckets (4)

    ei = np.concatenate([np.asarray(edge_index), np.asarray(y_edge_index)], axis=1)
    src_g = ei[0].astype(np.int64)
    dst_g = ei[1].astype(np.int64)
    owner = dst_g // SH

    per_core = []
    counts = np.zeros((NC, NQ, NG), np.int64)
    for c in range(NC):
        sel = owner == c
        s = src_g[sel].astype(np.int32)
        d = (dst_g[sel] - c * SH).astype(np.int32)
        q = s // BUCK
        order = np.lexsort((s, d // P, q))
        s, d, q = s[order], d[order], q[order]
        counts[c] = np.zeros((NQ, NG), np.int64)
        np.add.at(counts[c], (q, d // P), 1)
        per_core.append((s, d, q))

    # uniform tiles per (bucket, group) = max over cores (SPMD instr stream)
    Tqg = np.ceil(counts.max(axis=0) / P).astype(np.int64)  # [NQ, NG]
    T2 = int(Tqg.sum())
    # bucket-major tile layout: tiles of (q, g) start at tile_start[q, g]
    tile_start = np.concatenate([[0], np.cumsum(Tqg.ravel())])[:-1].reshape(NQ, NG)

    x = np.asarray(x, np.float32)
    xpad = np.zeros((NPAD, IN), np.float32)
    xpad[:N] = x
    assert N < NPAD

    iota128 = np.tile(np.arange(P, dtype=np.float32), (P, 1)).astype(
        ml_dtypes.bfloat16)
    b_rep = np.tile(np.asarray(b, np.float32)[None, :], (P, 1))
    W32 = np.asarray(W, np.float32)

    in_maps = []
    for c in range(NC):
        s, d, q = per_core[c]
        g = d // P
        # rank of each edge inside its (q, g) run
        run_start = np.zeros((NQ, NG), np.int64)
        flat_counts = counts[c]
        run_start.ravel()[1:] = np.cumsum(flat_counts.ravel())[:-1]
        slot_in_run = np.arange(len(d)) - run_start[q, g]
        pos = tile_start[q, g] * P + slot_in_run
        idx16 = np.zeros(T2 * P, np.int16)              # pad -> row 0 (masked)
        dr2 = np.full(T2 * P, 2.0 * P, np.float32)      # pad -> no mask match
        idx16[pos] = (s - q * BUCK).astype(np.int16)
        dr2[pos] = (d - g * P).astype(np.float32)
        # wrap idxs: idx i -> partition i%16, column i//16; replicate to 128
        idxw = np.ascontiguousarray(idx16.reshape(T2 * P // 16, 16).T)  # [16, T2*8]
        idxw = np.tile(idxw, (8, 1))                                    # [128, T2*8]
        # per-node dst-run prefix offsets (layout metadata; device does hi-lo)
        deg_off = np.concatenate(
            [[0], np.cumsum(np.bincount(d, minlength=SH))]).astype(np.float32)
        xt = np.ascontiguousarray(xpad[c * SH:(c + 1) * SH].T)  # [IN, SH]
        in_maps.append({
            "xT": xt,
            "W": W32,
            "b_rep": b_rep,
            "iota128": iota128,
            "idxs": idxw,
            "dr2": np.ascontiguousarray(
                dr2.reshape(T2, P).T.astype(ml_dtypes.bfloat16)),
            "slo": np.ascontiguousarray(deg_off[:SH].reshape(NG, P).T),
            "shi": np.ascontiguousarray(deg_off[1:SH + 1].reshape(NG, P).T),
        })
    return in_maps, Tqg.tolist(), T2


# ---------------------------------------------------------------- device module
def build_module(cfg, Tqg, T2):
    import concourse.bass as bass
    import concourse.bacc as bacc
    import concourse.tile as tile
    import concourse.mybir as mybir

    NC, SH = cfg["NC"], cfg["SH"]
    NPAD = NC * SH
    NG = SH // P
    NQ = (NPAD + BUCK - 1) // BUCK
    Tqg = np.asarray(Tqg)
    tile_start = np.concatenate([[0], np.cumsum(Tqg.ravel())])[:-1].reshape(NQ, NG)
    bucket_t0 = [int(tile_start[q, 0]) for q in range(NQ)]
    bucket_t1 = [int(tile_start[q, NG - 1] + Tqg[q, NG - 1]) for q in range(NQ)]

    nc = bacc.Bacc("TRN2", target_bir_lowering=False, debug=False,
                   enable_asserts=False, num_devices=NC,
                   num_swdge_queues=4)

    dt = mybir.dt
    xT_d = nc.dram_tensor("xT", [IN, SH], dt.float32, kind="ExternalInput")
    W_d = nc.dram_tensor("W", [IN, LAT], dt.float32, kind="ExternalInput")
    brep_d = nc.dram_tensor("b_rep", [P, LAT], dt.float32, kind="ExternalInput")
    iota128_d = nc.dram_tensor("iota128", [P, P], dt.bfloat16,
                               kind="ExternalInput")
    idxs_d = nc.dram_tensor("idxs", [P, T2 * 8], dt.int16, kind="ExternalInput")
    dr2_d = nc.dram_tensor("dr2", [P, T2], dt.bfloat16, kind="ExternalInput")
    slo_d = nc.dram_tensor("slo", [P, NG], dt.float32, kind="ExternalInput")
    shi_d = nc.dram_tensor("shi", [P, NG], dt.float32, kind="ExternalInput")
    out_d = nc.dram_tensor("out", [SH, LAT], dt.float32, kind="ExternalOutput")

    AF = mybir.ActivationFunctionType
    OP = mybir.AluOpType

    with tile.TileContext(nc) as tc:
        with tc.tile_pool(name="res", bufs=1) as res, \
             tc.tile_pool(name="dram", bufs=1, space="DRAM") as dram:
            idxs_t = res.tile([P, T2 * 8], dt.int16)
            dr2_t = res.tile([P, T2], dt.bfloat16)
            iota128_t = res.tile([P, P], dt.bfloat16)
            W_t = res.tile([IN, LAT], dt.float32)
            Wb_t = res.tile([IN, LAT], dt.bfloat16)
            brep_t = res.tile([P, LAT], dt.float32)
            slo_t = res.tile([P, NG], dt.float32)
            shi_t = res.tile([P, NG], dt.float32)
            dinv_t = res.tile([P, NG], dt.float32)
            h128 = res.tile([P, NG * LAT], dt.float32)    # h' = dinv * (x@W)
            acc128 = res.tile([P, NG * LAT], dt.float32)  # aggregated messages
            warm = res.tile([P, 512], dt.float32)

            h_shard = dram.tile([SH, EPAD], dt.bfloat16)
            h_full = dram.tile([NPAD, EPAD], dt.bfloat16, addr_space="Shared")

            nc.sync.dma_start(idxs_t[:], idxs_d[:])
            nc.sync.dma_start(dr2_t[:], dr2_d[:])
            nc.sync.dma_start(iota128_t[:], iota128_d[:])
            nc.sync.dma_start(W_t[:], W_d[:])
            nc.sync.dma_start(brep_t[:], brep_d[:])
            nc.sync.dma_start(slo_t[:], slo_d[:])
            nc.sync.dma_start(shi_t[:], shi_d[:])

            # deg = shi - slo ; dinv = 1/sqrt(deg + 1)
            nc.vector.tensor_tensor(out=slo_t[:], in0=shi_t[:], in1=slo_t[:],
                                    op=OP.subtract)
            nc.scalar.activation(shi_t[:], slo_t[:], AF.Sqrt, bias=1.0)
            nc.vector.reciprocal(dinv_t[:], shi_t[:])
            nc.scalar.activation(Wb_t[:], W_t[:], AF.Copy)

            # ---------------- phase A: h' = dinv * (x @ W) ------------------
            with tc.tile_pool(name="xt", bufs=1) as xtp, \
                 tc.tile_pool(name="psA", bufs=2, space="PSUM") as psA:
                # dense dummy matmul burst: drives the PE HAM out of the cold
                # throttle window before the real matmul stream
                nc.vector.memset(warm[:], 1.0)
                pw = psA.tile([P, 512], dt.float32, tag="h")
                for _ in range(10):
                    nc.tensor.matmul(out=pw[:], lhsT=warm[:, :P],
                                     rhs=warm[:], start=True, stop=True)
                nc.scalar.activation(warm[:, :1], pw[:, :1], AF.Copy)

                xf_t = xtp.tile([IN, SH], dt.float32)
                xb_t = xtp.tile([IN, SH], dt.bfloat16)
                nc.sync.dma_start(xf_t[:], xT_d[:])
                nc.scalar.activation(xb_t[:], xf_t[:], AF.Copy)
                for g in range(NG):
                    ph = psA.tile([P, LAT], dt.float32, tag="h")
                    nc.tensor.matmul(out=ph[:], lhsT=xb_t[:, g * P:(g + 1) * P],
                                     rhs=Wb_t[:], start=True, stop=True)
                    nc.vector.tensor_tensor(
                        out=h128[:, g * LAT:(g + 1) * LAT],
                        in0=ph[:],
                        in1=dinv_t[:, g:g + 1].to_broadcast([P, LAT]),
                        op=OP.mult)

            # ------- pass 2: bulk-gather + aggregate (128-node groups) -----
            with tc.tile_pool(name="mask2", bufs=MASK_BUFS) as mp2, \
                 tc.tile_pool(name="gat", bufs=G_BUFS) as gp, \
                 tc.tile_pool(name="psG", bufs=8, space="PSUM") as psG:
                gtiles = {}
                masks2 = {}

                # chunk list per bucket: [t0, t1) tile ranges, single bucket
                chunk_of = {}
                chunks = []
                for q in range(NQ):
                    t = bucket_t0[q]
                    while t < bucket_t1[q]:
                        t1 = min(t + GCH, bucket_t1[q])
                        for tt in range(t, t1):
                            chunk_of[tt] = len(chunks)
                        chunks.append((q, t, t1))
                        t = t1

                def get_chunk(ci):
                    if ci not in gtiles:
                        q, t0, t1 = chunks[ci]
                        cw = t1 - t0
                        rows = min(BUCK, NPAD - q * BUCK)
                        gt = gp.tile([P, GCH * EPAD], dt.bfloat16, tag="g")
                        nc.gpsimd.dma_gather(
                            gt[:, :cw * EPAD]
                                .rearrange("p (t e) -> p t e", e=EPAD),
                            h_full[q * BUCK:q * BUCK + rows, :],
                            idxs_t[:, t0 * 8:t1 * 8],
                            cw * P, cw * P, EPAD,
                            queue_num=ci % 4,
                        )
                        gtiles[ci] = gt
                    return gtiles[ci]

                def get_mask(j):
                    if j not in masks2:
                        cw = min(MC, T2 - j * MC)
                        mt = mp2.tile([P, MC * P], dt.bfloat16, tag="m2")
                        nc.vector.tensor_tensor(
                            out=mt[:, :cw * P]
                                .rearrange("p (t f) -> p t f", t=cw),
                            in0=dr2_t[:, j * MC:j * MC + cw, None]
                                .to_broadcast([P, cw, P]),
                            in1=iota128_t[:, None, :].to_broadcast([P, cw, P]),
                            op=OP.is_equal)
                        masks2[j] = mt
                    return masks2[j]

                # pre-build the first mask chunk of each bucket region while
                # the AllGather is in flight (masks depend only on dr2/iota)
                for q in range(NQ):
                    mj = int(tile_start[q, 0]) // MC
                    get_mask(mj)
                    if mj + 1 < (T2 + MC - 1) // MC:
                        get_mask(mj + 1)

                # ship h' shard (cast fp32->bf16) into cols :32 of the padded
                # rows (pad bytes never read downstream), AllGather the table
                nc.gpsimd.dma_start(
                    out=h_shard[:].rearrange("(g p) e -> p g e", p=P)[:, :, :LAT],
                    in_=h128[:].rearrange("p (g f) -> p g f", f=LAT))
                nc.gpsimd.collective_compute(
                    "AllGather", OP.bypass,
                    replica_groups=[list(range(NC))],
                    ins=[h_shard[:]], outs=[h_full[:]])

                for g in range(NG):
                    tlist = []
                    for q in range(NQ):
                        t0 = int(tile_start[q, g])
                        tlist.extend(range(t0, t0 + int(Tqg[q, g])))
                    if not tlist:
                        continue
                    pa = psG.tile([P, LAT], dt.float32, tag="agg")
                    for k, t in enumerate(tlist):
                        ci = chunk_of[t]
                        _, ct0, _ = chunks[ci]
                        mj, mo = divmod(t, MC)
                        nc.tensor.matmul(
                            out=pa[:],
                            lhsT=get_mask(mj)[:, mo * P:(mo + 1) * P],
                            rhs=get_chunk(ci)[
                                :, (t - ct0) * EPAD:(t - ct0) * EPAD + LAT],
                            start=(k == 0), stop=(k == len(tlist) - 1))
                    nc.scalar.activation(acc128[:, g * LAT:(g + 1) * LAT],
                                         pa[:], AF.Copy)

            # ---------------- finalize: out = dinv*(acc + h') + b ----------
            nc.vector.tensor_tensor(out=acc128[:], in0=acc128[:], in1=h128[:],
                                    op=OP.add)
            nc.vector.tensor_tensor(
                out=acc128[:].rearrange("p (g f) -> p g f", f=LAT),
                in0=acc128[:].rearrange("p (g f) -> p g f", f=LAT),
                in1=dinv_t[:, :, None].to_broadcast([P, NG, LAT]),
                op=OP.mult)
            nc.vector.tensor_tensor(
                out=acc128[:].rearrange("p (g f) -> p g f", f=LAT),
                in0=acc128[:].rearrange("p (g f) -> p g f", f=LAT),
                in1=brep_t[:, None, :].to_broadcast([P, NG, LAT]),
                op=OP.add)
            nc.sync.dma_start(
                out_d.rearrange("(g p) f -> p g f", p=P),
                acc128[:].rearrange("p (g f) -> p g f", f=LAT))

    nc.compile()
    return nc


# ---------------------------------------------------------------- entry point
LAST_EXEC_NS = None


def kernel(x, edge_index, y_edge_index, W, b):
    import os
    global LAST_EXEC_NS
    from concourse import bass_utils

    cfg = _full_cfg()
    in_maps, Tqg, T2 = prepare(x, edge_index, y_edge_index, W, b, cfg)
    nc = build_module(cfg, Tqg, T2)
    trace = os.environ.get("KERNEL_TRACE", "0") == "1"
    res = bass_utils.run_bass_kernel_spmd(nc, in_maps,
                                          core_ids=list(range(cfg["NC"])),
                                          trace=trace)
    if trace:
        LAST_EXEC_NS = res.exec_time_ns
        print("exec_time_ns:", res.exec_time_ns, flush=True)
    outs = [res.results[c]["out"] for c in range(cfg["NC"])]
    return np.concatenate(outs, axis=0)[:cfg["N"]].astype(np.float32)
